# revision 12
# baseline (speedup 1.0000x reference)
"""Multi-head cross-attention on 8 Trainium2 NeuronCores — v2.

Shapes (hardcoded): B=4, Ld=1024, Le=2048, d_model=1024, 8 heads x 128.
Sharding: core c handles batch b=c//2 and head-group g=c%2 (4 heads each).
Host sums the two partial output projections per batch and adds the
constant row bv@Wo^T + b_o.

Key structure:
- Softmax denominator is free: the attn@V matmul uses exp(S) chunks as the
  STATIONARY operand and [V | ones] (bf16) as the moving operand, so the
  PSUM accumulator's 129th column accumulates sum_k exp(s[k,q]) alongside
  the values. No ones-column matmuls.
- attn@V output lands as [q, vd]; normalization is a per-partition
  reciprocal multiply during the PSUM->SBUF copy; a cheap PE transpose
  restores [vd, q] for the output projection.
- Encoder is streamed in 8 chunks of 256 keys, one contiguous DMA each
  (host pre-interleaves all inputs into exact SBUF layouts, bf16), K+V
  projected per chunk in one pass.
- The QK+exp of the first query half's pairs is interleaved into the tail
  of the projection phase (kT key-chunks become valid per enc chunk), so
  the scalar engine's exp stream — the phase-2 pacer — starts ~30us early
  and the attention phase is PE-bound nearly throughout.
- One PSUM accumulation group per 2KB bank at a time (the matmul start
  flag zeroes the whole bank's zero-region).
"""

import math
import sys

import numpy as np

for _p in ("/opt/trn_rl_repo", "/root/.axon_site/_ro/trn_rl_repo"):
    if _p not in sys.path:
        sys.path.append(_p)

B = 4
LQ = 1024
LK = 2048
D = 1024
H = 8
DH = 128
P = 128
HPC = 4          # heads per core
OQ = HPC * DH    # 512 projected dims per core
KC = D // P      # 8 contraction chunks
N_CORES = 8
NEC = 8          # encoder stream chunks
EK = LK // NEC   # 256 keys per chunk
NXC = 4          # x stream chunks
XQ = LQ // NXC   # 256 queries per chunk
NKJ = LK // P    # 16 key chunks of 128 for attention

_BUILT = {}
DEBUG = False


def _build(masked):
    import concourse.bass as bass  # noqa: F401
    import concourse.tile as tile
    import concourse.mybir as mybir
    from concourse import bacc

    f32 = mybir.dt.float32
    f32r = mybir.dt.float32r
    bf16 = mybir.dt.bfloat16
    Exp = mybir.ActivationFunctionType.Exp
    Copy = mybir.ActivationFunctionType.Copy
    Ident = mybir.ActivationFunctionType.Identity

    nc = bacc.Bacc("TRN2", target_bir_lowering=False, debug=False,
                   num_devices=N_CORES)

    # Host-pre-interleaved inputs: first dim is the SBUF partition.
    enc_r = nc.dram_tensor("enc_r", [P, NEC, KC, EK], bf16,
                           kind="ExternalInput").ap()
    x_r = nc.dram_tensor("x_r", [P, NXC, KC, XQ], bf16,
                         kind="ExternalInput").ap()
    wk_r = nc.dram_tensor("wk_r", [P, KC, OQ], bf16, kind="ExternalInput").ap()
    wv_r = nc.dram_tensor("wv_r", [P, KC, OQ], bf16, kind="ExternalInput").ap()
    wq_r = nc.dram_tensor("wq_r", [P, KC, OQ], bf16, kind="ExternalInput").ap()
    wo_r = nc.dram_tensor("wo_r", [P, HPC, D], bf16, kind="ExternalInput").ap()
    bq_d = nc.dram_tensor("bq", [P, HPC], f32, kind="ExternalInput").ap()
    bk_d = nc.dram_tensor("bk", [P, HPC], f32, kind="ExternalInput").ap()
    id_d = nc.dram_tensor("ident", [P, P], bf16, kind="ExternalInput").ap()
    if masked:
        # mask_r[p, q2, t, jj, q] = mask[(2t+jj)*128+p, q2*512+q]
        mask_r = nc.dram_tensor("mask_r", [P, 2, 8, 2, LQ // 2], f32,
                                kind="ExternalInput").ap()
    out_d = nc.dram_tensor("out", [LQ, D], bf16, kind="ExternalOutput").ap()

    with tile.TileContext(nc) as tc:
        with (
            tc.tile_pool(name="persist", bufs=1) as persist,
            tc.tile_pool(name="xpool", bufs=2) as xpool,
            tc.tile_pool(name="ps_s", bufs=2, space="PSUM") as pss,
            tc.tile_pool(name="pTp", bufs=36 if not masked else 10) as pTp,
            tc.tile_pool(name="vstage", bufs=8) as vstage,
            tc.tile_pool(name="rp", bufs=8) as rp,
            tc.tile_pool(name="osb", bufs=6) as osb,
            tc.tile_pool(name="maskp", bufs=6 if masked else 1) as maskp,
        ):
            wk_sb = persist.tile([P, KC, OQ], bf16, name="wk")
            wv_sb = persist.tile([P, KC, OQ], bf16, name="wv")
            wq_sb = persist.tile([P, KC, OQ], bf16, name="wq")
            wo_sb = persist.tile([P, HPC, D], bf16, name="wo")
            kT = [persist.tile([P, LK], bf16, name=f"kT{h}")
                  for h in range(HPC)]
            qT = [persist.tile([P, LQ], bf16, name=f"qT{h}")
                  for h in range(HPC)]
            v1 = [persist.tile([P, HPC, DH + 1], bf16, name=f"v1_{j}")
                  for j in range(NKJ)]
            valsT = [persist.tile([P, LQ], bf16, name=f"valsT{h}")
                     for h in range(HPC)]
            bq_sb = persist.tile([P, HPC], f32, name="bq")
            bk_sb = persist.tile([P, HPC], f32, name="bk")
            id_sb = persist.tile([P, P], bf16, name="ident")

            xch = [None] * NXC
            pairs = [(q2, h) for q2 in (0, 1) for h in range(HPC)]
            pts_of = {}

            def qproj(qq, pool, qname, hs=range(HPC)):
                xq = xch[qq]
                for h in hs:
                    qb = pool.tile([P, 512], f32, name=qname)
                    for d in range(KC):
                        nc.tensor.matmul(
                            qb[:, 0:XQ],
                            wq_sb[:, d, h * DH:(h + 1) * DH],
                            xq[:, d, :],
                            start=(d == 0), stop=(d == KC - 1))
                    nc.vector.tensor_scalar_add(
                        qT[h][:, qq * XQ:(qq + 1) * XQ],
                        qb[:, 0:XQ], bq_sb[:, h:h + 1])

            def qk_dual(i, t):
                """QK + exp for dual key-chunk t of pair i."""
                q2, h = pairs[i]
                qsl = qT[h][:, q2 * 512:(q2 + 1) * 512]
                ss = pss.tile([P, 1024], f32, name="ss")
                for jj in range(2):
                    j = 2 * t + jj
                    nc.tensor.matmul(
                        ss[:, jj * 512:(jj + 1) * 512],
                        kT[h][:, j * P:(j + 1) * P],
                        qsl,
                        start=True, stop=True)
                if masked:
                    mt = maskp.tile([P, 1024], f32, name="mt")
                    nc.sync.dma_start(mt[:], mask_r[:, q2, t])
                    nc.vector.tensor_add(ss[:], ss[:], mt[:])
                pt = pTp.tile([P, 1024], bf16, name="pt")
                nc.scalar.activation(pt[:], ss[:], Exp)
                pts_of.setdefault(i, []).append(pt)

            # ---- Phase 1: stream enc; K+V projection per 256-key chunk;
            # Q projection and the first query half's QK/exp interleaved
            # into the tail.
            with (
                tc.tile_pool(name="ps1", bufs=2, space="PSUM") as ps1,
                tc.tile_pool(name="epool", bufs=3) as epool,
            ):
                for j in range(NKJ):
                    nc.vector.memset(v1[j][:, :, DH:DH + 1], 1.0)

                # p-state warmup: keep the PE continuously busy on zeroed
                # scratch during the cold DMA wait so the clock is at full
                # speed when real data lands (the ramp needs 3us of
                # uninterrupted execution).
                scr = persist.tile([P, 640], bf16, name="warm")
                nc.vector.memset(scr[:], 0.0)

                def warm(n):
                    for _ in range(n):
                        wb = ps1.tile([P, 512], f32, name="kb")
                        nc.tensor.matmul(wb[:], scr[:, 0:P], scr[:, P:640],
                                         start=True, stop=True)

                warm(6)

                # cold-start friendly DMA order: small first pieces so the
                # first matmuls start ASAP, then interleave bulk behind the
                # enc stream.
                ech = []
                e0 = epool.tile([P, KC, EK], bf16, name="e")
                nc.sync.dma_start(wk_sb[:, 0:2], wk_r[:, 0:2])
                nc.scalar.dma_start(e0[:, 0:2], enc_r[:, 0, 0:2])
                nc.sync.dma_start(wk_sb[:, 2:5], wk_r[:, 2:5])
                nc.scalar.dma_start(e0[:, 2:5], enc_r[:, 0, 2:5])
                nc.sync.dma_start(wk_sb[:, 5:8], wk_r[:, 5:8])
                nc.scalar.dma_start(e0[:, 5:8], enc_r[:, 0, 5:8])
                ech.append(e0)
                nc.sync.dma_start(bk_sb[:], bk_d[:])
                nc.sync.dma_start(wv_sb[:, 0:4], wv_r[:, 0:4])
                e1 = epool.tile([P, KC, EK], bf16, name="e")
                nc.sync.dma_start(e1[:, 0:4], enc_r[:, 1, 0:4])
                nc.sync.dma_start(wv_sb[:, 4:8], wv_r[:, 4:8])
                nc.sync.dma_start(e1[:, 4:8], enc_r[:, 1, 4:8])
                ech.append(e1)
                nc.sync.dma_start(bq_sb[:], bq_d[:])
                nc.sync.dma_start(id_sb[:], id_d[:])
                for c in range(2, NEC):
                    e = epool.tile([P, KC, EK], bf16, name="e")
                    nc.sync.dma_start(e[:], enc_r[:, c])
                    ech.append(e)
                    if c == 2:
                        nc.sync.dma_start(wq_sb[:], wq_r[:])
                    elif c == 3:
                        xch[0] = xpool.tile([P, KC, XQ], bf16, name="xq")
                        nc.sync.dma_start(xch[0][:], x_r[:, 0])
                    elif c == 4:
                        xch[1] = xpool.tile([P, KC, XQ], bf16, name="xq")
                        nc.sync.dma_start(xch[1][:], x_r[:, 1])
                    elif c == 5:
                        nc.sync.dma_start(wo_sb[:], wo_r[:])

                def kproj(c):
                    e = ech[c]
                    # one accumulation group per PSUM bank at a time: the
                    # start flag zeroes the whole 2KB zero-region.
                    for h in range(HPC):
                        kb = ps1.tile([P, 512], f32, name="kb")
                        for d in range(KC):
                            nc.tensor.matmul(
                                kb[:, 0:EK],
                                wk_sb[:, d, h * DH:(h + 1) * DH],
                                e[:, d, :],
                                start=(d == 0), stop=(d == KC - 1))
                        nc.scalar.activation(
                            kT[h][:, c * EK:(c + 1) * EK],
                            kb[:, 0:EK],
                            Ident, bias=bk_sb[:, h:h + 1])

                def vproj(c):
                    e = ech[c]
                    for kw in range(2):
                        vb = ps1.tile([P, 512], f32, name="vb")
                        for d in range(KC):
                            nc.tensor.matmul(
                                vb[:],
                                e[:, d, kw * P:(kw + 1) * P],
                                wv_sb[:, d, :],
                                start=(d == 0), stop=(d == KC - 1))
                        j = 2 * c + kw
                        nc.scalar.activation(
                            v1[j][:, :, 0:DH],
                            vb[:].rearrange("p (h d) -> p h d", h=HPC),
                            Copy)

                # duals of the q2=0 pairs emitted inside phase 1, per enc
                # chunk (dual (p, t) needs kproj(t) and qproj(0..1) done).
                dual_sched = {
                    4: [(0, 0), (0, 1), (0, 2), (0, 3)],
                    5: [(0, 4), (0, 5), (1, 0), (1, 1), (1, 2), (1, 3)],
                    6: [(0, 6), (1, 4), (1, 5), (1, 6),
                        (2, 0), (2, 1), (2, 2), (2, 3)],
                    7: [(0, 7), (1, 7), (2, 4), (2, 5), (2, 6), (2, 7),
                        (3, 0), (3, 1), (3, 2), (3, 3)],
                } if not masked else {}

                def kproj_douter(c):
                    # chunk 0 is DMA-paced: d-outer over 4 parallel banks
                    # consumes each wk/e piece as it lands.
                    e = ech[c]
                    tiles = [ps1.tile([P, 512], f32, name="kb"),
                             ps1.tile([P, 512], f32, name="kb"),
                             ps1.tile([P, 512], f32, name="vb"),
                             ps1.tile([P, 512], f32, name="vb")]
                    for d in range(KC):
                        for h in range(HPC):
                            nc.tensor.matmul(
                                tiles[h][:, 0:EK],
                                wk_sb[:, d, h * DH:(h + 1) * DH],
                                e[:, d, :],
                                start=(d == 0), stop=(d == KC - 1))
                    for h in range(HPC):
                        nc.scalar.activation(
                            kT[h][:, c * EK:(c + 1) * EK],
                            tiles[h][:, 0:EK],
                            Ident, bias=bk_sb[:, h:h + 1])

                for c in range(NEC):
                    if c == NEC - 1 and not masked:
                        # qT half 1 before the last enc chunk: the Act FIFO
                        # is deep in queued exps here, so waiting on chunk
                        # 7's PSUM-ring copies would stall the PE.
                        qproj(2, ps1, "kb")
                        qproj(3, ps1, "vb")
                    if c == 0:
                        kproj_douter(c)
                    else:
                        kproj(c)
                    vproj(c)
                    if c == 2:
                        qproj(0, ps1, "kb")
                        xch[2] = xpool.tile([P, KC, XQ], bf16, name="xq")
                        nc.sync.dma_start(xch[2][:], x_r[:, 2])
                    elif c == 3:
                        qproj(1, ps1, "kb")
                        xch[3] = xpool.tile([P, KC, XQ], bf16, name="xq")
                        nc.sync.dma_start(xch[3][:], x_r[:, 3])
                    for (p, t) in dual_sched.get(c, []):
                        qk_dual(p, t)

                if not masked:
                    for (p, t) in [(3, 4), (3, 5), (3, 6), (3, 7)]:
                        qk_dual(p, t)

            # ---- Phase 2: attention + output projection.
            with (
                tc.tile_pool(name="pv_ps", bufs=2, space="PSUM") as pvps,
                tc.tile_pool(name="to_ps", bufs=2, space="PSUM") as tops,
            ):
                if masked:
                    xch[2] = xpool.tile([P, KC, XQ], bf16, name="xq")
                    nc.sync.dma_start(xch[2][:], x_r[:, 2])
                    qproj(2, tops, "t")
                    xch[3] = xpool.tile([P, KC, XQ], bf16, name="xq")
                    nc.sync.dma_start(xch[3][:], x_r[:, 3])
                    qproj(3, tops, "t")

                def pv_ops(i, use_pe_transpose=False, lag_trans=True):
                    """Emission thunks for pair i's attn@V+norm+transpose:
                    a list of 8 chunks, each a list of callables."""
                    q2, h = pairs[i]
                    pts = pts_of[i]
                    pvt_tiles = {}

                    def pv_mm(pvt, qc2, j):
                        def go():
                            if pvt not in pvt_tiles:
                                pvt_tiles[pvt] = pvps.tile(
                                    [P, 2 * (DH + 1)], f32, name="pv")
                            pv = pvt_tiles[pvt]
                            qc = pvt * 2 + qc2
                            off = qc2 * (DH + 1)
                            nc.tensor.matmul(
                                pv[:, off:off + DH + 1],
                                pts[j // 2][:, (j % 2) * 512 + qc * P:
                                            (j % 2) * 512 + qc * P + P],
                                v1[j][:, h, :],
                                start=(j == 0), stop=(j == NKJ - 1))
                        return go

                    vts = {}

                    def norm(pvt, qc2):
                        def go():
                            pv = pvt_tiles[pvt]
                            qc = pvt * 2 + qc2
                            off = qc2 * (DH + 1)
                            recip = rp.tile([P, 1], f32, name="recip")
                            nc.vector.reciprocal(
                                recip[:], pv[:, off + DH:off + DH + 1])
                            vt = vstage.tile([P, P], bf16, name="vt")
                            nc.vector.tensor_scalar_mul(
                                vt[:], pv[:, off:off + DH], recip[:])
                            vts[qc] = vt
                        return go

                    def trans(qc):
                        def go():
                            dst = valsT[h][:, q2 * 512 + qc * P:
                                           q2 * 512 + (qc + 1) * P]
                            if use_pe_transpose:
                                tpf = tops.tile([P, 512], f32, name="t")
                                tp = tpf[:, 0:64].bitcast(bf16)
                                nc.tensor.transpose(tp, vts[qc][:], id_sb[:])
                                nc.vector.tensor_copy(dst, tp)
                            else:
                                # xbar transpose on the (phase-2 idle) DMA
                                # engines writes valsT directly.
                                nc.sync.dma_start_transpose(dst, vts[qc][:])
                        return go

                    # transpose+copy lag the recip+mul by one chunk so the
                    # PE never waits on the DVE normalize chain (unless the
                    # caller needs each qc's valsT available chunk-tight).
                    chunks = []
                    for pvt in range(2):
                        for qc2 in range(2):
                            qc = pvt * 2 + qc2
                            ops = [pv_mm(pvt, qc2, j) for j in range(NKJ)]
                            ops.append(norm(pvt, qc2))
                            a, b = ops[:8], ops[8:]
                            if lag_trans:
                                if qc > 0:
                                    a = [trans(qc - 1)] + a
                            else:
                                b.append(trans(qc))
                            chunks.append(a)
                            chunks.append(b)
                    if lag_trans:
                        chunks[7].append(trans(3))
                    return chunks

                def o_group(q2, lqc, o2, copy_act=False):
                    def go():
                        po = tops.tile([P, 512], f32, name="t")
                        for h in range(HPC):
                            nc.tensor.matmul(
                                po[:],
                                valsT[h][:, lqc * P:(lqc + 1) * P],
                                wo_sb[:, h, o2 * 512:(o2 + 1) * 512],
                                start=(h == 0), stop=(h == HPC - 1))
                        ot = osb.tile([P, 512], bf16, name="ot")
                        if copy_act:
                            # drain: Act is idle and DVE carries the PV
                            # finalize chain — keep the store copy off DVE.
                            nc.scalar.activation(ot[:], po[:], Copy)
                        else:
                            nc.vector.tensor_copy(ot[:], po[:])
                        nc.sync.dma_start(
                            out_d[lqc * P:(lqc + 1) * P,
                                  o2 * 512:(o2 + 1) * 512], ot[:])
                    return go

                o0 = [o_group(0, lqc, o2)
                      for lqc in range(4) for o2 in range(2)]
                o1 = [o_group(1, lqc, o2, copy_act=True)
                      for lqc in range(4, 8) for o2 in range(2)]

                if not masked:
                    # q2=0 exps all ran in phase 1; phase 2 is PE-bound:
                    # s0: QK(p4) + PV(p0) + PV(p1)
                    # s1: QK(p5) + PV(p2) + PV(p3)
                    # s2: QK(p6) + O0[0:6]
                    # s3: QK(p7) + PV(p4) + O0[6:8]
                    # s4: PV(p5) + PV(p6)
                    # s5: PV(p7) + O1
                    pva = pv_ops(0)
                    pvb = pv_ops(1)
                    for t in range(8):
                        for op in pva[t]:
                            op()
                        qk_dual(4, t)
                        for op in pvb[t]:
                            op()
                    pva = pv_ops(2)
                    pvb = pv_ops(3)
                    for t in range(8):
                        qk_dual(5, t)
                        for op in pva[t]:
                            op()
                        for op in pvb[t]:
                            op()
                    for t in range(8):
                        if t < 6:
                            o0[t]()
                        qk_dual(6, t)
                    pva = pv_ops(4)
                    for t in range(8):
                        if t < 2:
                            o0[6 + t]()
                        qk_dual(7, t)
                        for op in pva[t]:
                            op()
                    pva = pv_ops(5)
                    pvb = pv_ops(6)
                    for t in range(8):
                        for op in pva[t]:
                            op()
                        for op in pvb[t]:
                            op()
                    # pair 7 drain, inlined: o-groups of qc-1 run inside
                    # the norm(qc) DVE-latency window so the PE never waits
                    # on the recip+mul chain before each transpose.
                    q2h, hh7 = pairs[7]
                    pts7 = pts_of[7]
                    for pvt in range(2):
                        pv = pvps.tile([P, 2 * (DH + 1)], f32, name="pv")
                        for qc2 in range(2):
                            qc = pvt * 2 + qc2
                            off = qc2 * (DH + 1)
                            for j in range(NKJ):
                                nc.tensor.matmul(
                                    pv[:, off:off + DH + 1],
                                    pts7[j // 2][:, (j % 2) * 512 + qc * P:
                                                 (j % 2) * 512 + qc * P + P],
                                    v1[j][:, hh7, :],
                                    start=(j == 0), stop=(j == NKJ - 1))
                            recip = rp.tile([P, 1], f32, name="recip")
                            nc.vector.reciprocal(
                                recip[:], pv[:, off + DH:off + DH + 1])
                            vt = vstage.tile([P, P], bf16, name="vt")
                            nc.vector.tensor_scalar_mul(
                                vt[:], pv[:, off:off + DH], recip[:])
                            if qc > 0:
                                o1[2 * (qc - 1)]()
                                o1[2 * (qc - 1) + 1]()
                            tpf = tops.tile([P, 512], f32, name="t")
                            tp = tpf[:, 0:64].bitcast(bf16)
                            nc.tensor.transpose(tp, vt[:], id_sb[:])
                            nc.vector.tensor_copy(
                                valsT[hh7][:, q2h * 512 + qc * P:
                                           q2h * 512 + (qc + 1) * P],
                                tp)
                    o1[6]()
                    o1[7]()
                else:
                    # masked path: sequential pairs, QK/exp + mask add per
                    # dual, PV of pair i-1 interleaved under pair i.
                    for i in range(len(pairs)):
                        pv_chunks = (pv_ops(i - 1, use_pe_transpose=True)
                                     if i > 0 else None)
                        for t in range(8):
                            qk_dual(i, t)
                            if pv_chunks:
                                for op in pv_chunks[t]:
                                    op()
                        if i == 4:
                            for g in o0:
                                g()
                    pv_chunks = pv_ops(len(pairs) - 1, use_pe_transpose=True)
                    for t in range(8):
                        for op in pv_chunks[t]:
                            op()
                        if t >= 3:
                            o1[t - 3]()
                    o1[5]()
                    o1[6]()
                    o1[7]()

    nc.compile()
    return nc


def _get_built(masked):
    if masked not in _BUILT:
        _BUILT[masked] = _build(masked)
    return _BUILT[masked]


def _shard_inputs(inputs, masked):
    import ml_dtypes

    x = np.asarray(inputs["mhca_input"], np.float32)
    enc = np.asarray(inputs["encoder_output"], np.float32)
    mask = np.asarray(inputs["cross_mask"], np.float32)
    W_kv = np.asarray(inputs["W_kv"], np.float32)
    b_kv = np.asarray(inputs["b_kv"], np.float32)
    W_q = np.asarray(inputs["W_q"], np.float32)
    b_q = np.asarray(inputs["b_q"], np.float32)
    W_o = np.asarray(inputs["W_o"], np.float32)

    scale = 1.0 / math.sqrt(DH)
    ident = np.eye(P, dtype=ml_dtypes.bfloat16)
    in_maps = []
    for c in range(N_CORES):
        b = c // 2
        g = c % 2
        heads = list(range(g * HPC, (g + 1) * HPC))
        sl = slice(g * OQ, (g + 1) * OQ)
        # K/V rows of W_kv for this core's heads, head-major [512, 1024]
        k_rows = np.concatenate(
            [W_kv[h * 2 * DH:h * 2 * DH + DH] for h in heads], 0)
        v_rows = np.concatenate(
            [W_kv[h * 2 * DH + DH:(h + 1) * 2 * DH] for h in heads], 0)
        bft = ml_dtypes.bfloat16
        # weight layouts [P, KC, OQ]: w_r[p, d, hc] = W.T[d*128+p, hc]
        wk = np.ascontiguousarray(
            k_rows.T.reshape(KC, P, OQ).transpose(1, 0, 2).astype(bft))
        wv = np.ascontiguousarray(
            v_rows.T.reshape(KC, P, OQ).transpose(1, 0, 2).astype(bft))
        wq = np.ascontiguousarray(
            (W_q[sl] * scale).T.reshape(KC, P, OQ).transpose(1, 0, 2)
            .astype(bft))
        # wo_r[p, h, o] = W_o[:, sl].T[h*128+p, o]
        wo = np.ascontiguousarray(
            W_o[:, sl].T.reshape(HPC, P, D).transpose(1, 0, 2).astype(bft))
        # enc_r[p, c, d, k] = enc[b][c*EK+k, d*128+p]
        er = np.ascontiguousarray(
            enc[b].reshape(NEC, EK, KC, P).transpose(3, 0, 2, 1).astype(bft))
        # x_r[p, qq, d, q] = x[b][qq*XQ+q, d*128+p]
        xr = np.ascontiguousarray(
            x[b].reshape(NXC, XQ, KC, P).transpose(3, 0, 2, 1).astype(bft))
        m = {
            "enc_r": er,
            "x_r": xr,
            "wk_r": wk,
            "wv_r": wv,
            "wq_r": wq,
            "wo_r": wo,
            "bq": np.ascontiguousarray((b_q[sl] * scale).reshape(HPC, DH).T),
            "bk": np.ascontiguousarray(
                np.stack([b_kv[h * 2 * DH:h * 2 * DH + DH] for h in heads],
                         1)),
            "ident": ident,
        }
        if masked:
            # mask_r[p, q2, t, jj, q] = mask[b][q2*512+q, (2t+jj)*128+p]
            m["mask_r"] = np.ascontiguousarray(
                mask[b].T.reshape(8, 2, P, 2, LQ // 2).transpose(
                    2, 3, 0, 1, 4))
        in_maps.append(m)
    return in_maps


def kernel(mhca_input, encoder_output, cross_mask, W_kv, b_kv, W_q, b_q, W_o,
           b_o):
    from concourse.bass_utils import run_bass_kernel_spmd

    inputs = {
        "mhca_input": mhca_input, "encoder_output": encoder_output,
        "cross_mask": cross_mask, "W_kv": W_kv, "b_kv": b_kv, "W_q": W_q,
        "b_q": b_q, "W_o": W_o,
    }
    b_o = np.asarray(b_o, np.float32)
    b_kv = np.asarray(b_kv, np.float32)
    W_o = np.asarray(W_o, np.float32)
    masked = bool(np.any(np.asarray(cross_mask)))
    nc = _get_built(masked)
    in_maps = _shard_inputs(inputs, masked)

    res = run_bass_kernel_spmd(nc, in_maps, core_ids=list(range(N_CORES)))
    outs = [np.asarray(res.results[c]["out"], np.float32)
            for c in range(N_CORES)]
    full = np.stack([outs[2 * b] + outs[2 * b + 1] for b in range(B)], 0)
    # host epilogue: attn@(V+bv) @ Wo^T + b_o = device partials
    #   + (bv_vals_space @ Wo^T + b_o), with bv in vals space h*128+d.
    bv = np.concatenate(
        [b_kv[h * 2 * DH + DH:(h + 1) * 2 * DH] for h in range(H)], 0)
    corr = bv @ W_o.T + b_o
    return (full + corr[None, None, :]).astype(np.float32)
